# revision 18
# baseline (speedup 1.0000x reference)
"""Causal self-attention on 8 trn2 NeuronCores.

Sharding: core = (batch b, head-group g) with b in 0..3, g in 0..1.
Each core computes, for its batch and its 8 heads (512 of 1024 embed dims):
  QT/KT projections stored transposed [e', s] (e' on partitions)
  V stored [s, e'] with a ones-column appended per head
  S^T[k, q] = K_h Q_h^T      (scores transposed; k on partitions)
  P^T = exp(S^T / 8)         (no max-subtraction; scores are O(1))
  causal zeroing of P^T via gpsimd affine_select on the 128-col
  triangle subtile of diagonal tiles
  att'^T[d, q] = sum_k V'_h[k, d] P^T[k, q]   (row 64 = softmax denom l)
  att_n^T = att'^T[0:64] * (1/l)  (gpsimd partition_broadcast of 1/l)
  out_partial = att_n^T.T @ Wo[rows_g, :]
Host sums the two g-partials per batch (partials are bf16; sum in f32).

All matmuls run in bfloat16.  x is cast to bf16 on the host, packed so
every input DMA moves >=2KB per partition row (8KB for x/wv/wo), and
kept resident in SBUF.

Scheduling: per kt step the two heads' score matmuls share ONE
standalone full-row LDWEIGHTS (their stacked kT is contiguous in SBUF;
the matmuls are emitted non-self-loading) and write the 2 banks of one
[128, 2, 512] PSUM tile; a SINGLE 1024-wide ACTIVATE exps both heads
(amortizing the ~300ns ACT fixed cost 2x, full width -- stale
diag-triangle PSUM columns are exp'd but never read).  The s_ps ring
stays 2-deep (4 banks) so the Scalar engine never idles waiting for
score matmuls.  Projection / output-projection matmul groups are
interleaved INTO the attention stream by a deficit-paced scheduler
that models PE and ACT clocks; a few output-projection groups are held
back as filler for the final (filler-starved) diagonal block.  The AV
matmuls run two kt behind their score matmuls (software pipelining) so
the in-order PE stream never blocks on a just-issued exp.
"""
import sys

if "/opt/trn_rl_repo" not in sys.path:
    sys.path.insert(0, "/opt/trn_rl_repo")

import numpy as np
import ml_dtypes

import concourse.bacc as bacc
import concourse.mybir as mybir
import concourse.tile as tile
from concourse.bass_utils import run_bass_kernel_spmd

S = 2048          # sequence length
E = 1024          # embed dim
G = 512           # per-core head-group width (8 heads x 64)
HD = 64           # head dim
NH = 8            # heads per core
EC = E // 128     # 8 E-chunks
ST = S // 128     # 16 s-tiles
SB = S // 512     # 4 s-blocks
F32 = mybir.dt.float32
BF16 = mybir.dt.bfloat16
EXP = mybir.ActivationFunctionType.Exp
GE = mybir.AluOpType.is_ge

_CACHE = {}


def _emit(nc, tc):
    xp = nc.declare_dram_parameter("xp", [SB, 128, EC, 512], BF16,
                                   isOutput=False)
    wq = nc.declare_dram_parameter("wq", [4, 128, EC, 128], BF16,
                                   isOutput=False)
    wk = nc.declare_dram_parameter("wk", [4, 128, EC, 128], BF16,
                                   isOutput=False)
    wv = nc.declare_dram_parameter("wv", [128, EC, 512], BF16,
                                   isOutput=False)
    wo = nc.declare_dram_parameter("wo", [128, 4, E], BF16, isOutput=False)
    c_ones = nc.declare_dram_parameter("c_ones", [128, NH], BF16,
                                       isOutput=False)
    out = nc.declare_dram_parameter("out", [ST, 128, E], BF16,
                                    isOutput=True)

    # ---- long-lived SBUF state ----
    persist1 = tc.alloc_tile_pool(name="persist1", bufs=1, side="right")
    xall = persist1.tile([128, SB, EC, 512], BF16, name="xall", tag="xall")
    qT_sb, kT_sb = [], []
    for c in range(4):
        qT_sb.append(persist1.tile([128, S], BF16, name=f"qT{c}",
                                   tag=f"qT{c}"))
        kT_sb.append(persist1.tile([128, S], BF16, name=f"kT{c}",
                                   tag=f"kT{c}"))
    vP = [persist1.tile([128, NH, HD + 1], BF16, name=f"vP{st}",
                        tag=f"vP{st}") for st in range(ST)]
    att_n = [persist1.tile([128, S], BF16, name=f"attn{c}", tag=f"attn{c}")
             for c in range(4)]
    ones_sb = persist1.tile([128, NH], BF16, name="ones_sb", tag="ones_sb")
    wq_all = persist1.tile([128, 4, EC, 128], BF16, name="wq_all",
                           tag="wq_all")
    wk_all = persist1.tile([128, 4, EC, 128], BF16, name="wk_all",
                           tag="wk_all")
    wv_all = persist1.tile([128, EC, 512], BF16, name="wv_all",
                           tag="wv_all")
    wo_all = persist1.tile([128, 4, E], BF16, name="wo_all", tag="wo_all")
    wqk_sb = {("q", c): wq_all[:, c] for c in range(4)}
    wqk_sb.update({("k", c): wk_all[:, c] for c in range(4)})

    # ---- input DMAs, ordered so early compute unblocks first; every
    # transfer moves >=2KB per partition row ----
    nc.sync.dma_start(out=wq_all[:, 0], in_=wq[0])
    nc.sync.dma_start(out=wk_all[:, 0], in_=wk[0])
    nc.sync.dma_start(out=xall[:, 0], in_=xp[0])
    nc.sync.dma_start(out=wv_all, in_=wv[:, :, :])
    nc.sync.dma_start(out=ones_sb, in_=c_ones[:, :])
    nc.sync.dma_start(out=xall[:, 1], in_=xp[1])
    for c in range(1, 4):
        nc.sync.dma_start(out=wq_all[:, c], in_=wq[c])
        nc.sync.dma_start(out=wk_all[:, c], in_=wk[c])
    nc.sync.dma_start(out=xall[:, 2], in_=xp[2])
    nc.sync.dma_start(out=xall[:, 3], in_=xp[3])
    nc.sync.dma_start(out=wo_all, in_=wo[:, :, :])

    # ---- PSUM pools: 2x2 (s_ps ring) + 2 (att) + 2 (proj) = 8 banks ----
    pst = tc.alloc_tile_pool(name="pst", bufs=2, space="PSUM")
    psatt = tc.alloc_tile_pool(name="psatt", bufs=2, space="PSUM")
    pp = tc.alloc_tile_pool(name="pp", bufs=2, space="PSUM")
    ptp = tc.alloc_tile_pool(name="ptp", bufs=4)
    smalls = tc.alloc_tile_pool(name="smalls", bufs=2)
    ostage = tc.alloc_tile_pool(name="ostage", bufs=2)

    # warm-up: junk matmuls on a zeroed scratch tile keep the PE busy
    # through the startup DMA window so the HAM clock gate reaches full
    # rate before the first real projection group (results unread;
    # the first real start=True group overwrites the psum banks)
    warm = persist1.tile([128, 512], BF16, name="warm", tag="warm")
    nc.vector.memset(warm, 0.0)
    for _ in range(14):
        wps = pp.tile([128, 512], F32, name="ps_proj", tag="ps_proj")
        nc.tensor.matmul(wps, lhsT=warm[:, 0:128], rhs=warm,
                         start=True, stop=True, skip_group_check=True)

    # ---- stall-driven scheduler ----
    # T["pe"] / T["act"] model the two engines' busy-until times over the
    # emitted in-order streams.  Filler (projection / output-projection
    # half-groups, ~870ns of PE work each) is emitted exactly where the
    # in-order PE stream would otherwise stall waiting for an exp.
    T = {"pe": 0.0, "act": 0.0}
    emitted = set()
    filler = []   # list of (uid, closure)

    def _mm(rows):
        T["pe"] += rows * 0.425

    def _act(width):
        T["act"] = max(T["act"], T["pe"]) + width * 0.833 + 295.0

    def _emit_next_filler():
        uid, fn = filler.pop(0)
        fn()
        emitted.add(uid)

    def flush_for(need):
        while need - emitted:
            _emit_next_filler()

    def fill_until(t):
        while T["pe"] < t and filler:
            _emit_next_filler()

    def _proj_halves(uid, lhs_of_ec, rhs_of_ec, finish):
        # one [128,512] psum accumulation group as two filler halves
        state = {}

        def fn_a():
            state["ps"] = pp.tile([128, 512], F32, name="ps_proj",
                                  tag="ps_proj")
            for ec in range(4):
                nc.tensor.matmul(
                    state["ps"], lhsT=lhs_of_ec(ec), rhs=rhs_of_ec(ec),
                    start=(ec == 0), stop=False, skip_group_check=True)
            _mm(4 * 512)

        def fn_b():
            for ec in range(4, EC):
                nc.tensor.matmul(
                    state["ps"], lhsT=lhs_of_ec(ec), rhs=rhs_of_ec(ec),
                    start=False, stop=(ec == EC - 1), skip_group_check=True)
            _mm(4 * 512)
            finish(state["ps"])
        return [(uid + ("a",), fn_a), (uid, fn_b)]

    def qk_units(c, sb_i):
        units = []
        for wname, dest in (("q", qT_sb), ("k", kT_sb)):
            def finish(ps, dest=dest):
                nc.vector.tensor_copy(
                    dest[c][:, sb_i * 512:(sb_i + 1) * 512], ps)
            units.extend(_proj_halves(
                ("qk", c, sb_i, wname),
                lambda ec, wname=wname: wqk_sb[(wname, c)][:, ec, :],
                lambda ec: xall[:, sb_i, ec, :],
                finish))
        return units

    def v_units(st):
        def finish(ps):
            nc.vector.tensor_copy(vP[st][:, :, 0:HD],
                                  ps.rearrange("p (h d) -> p h d", h=NH))
            nc.vector.tensor_copy(vP[st][:, :, HD], ones_sb)
        c0 = (st % 4) * 128
        return _proj_halves(
            ("v", st),
            lambda ec: xall[:, st // 4, ec, c0:c0 + 128],
            lambda ec: wv_all[:, ec],
            finish)

    def o_units(qb, s4):
        st = qb * 4 + s4
        state = {}
        units = []
        for eb in range(2):
            def fn(eb=eb):
                ps = pp.tile([128, 512], F32, name="ps_o", tag="ps_proj")
                for c in range(4):
                    nc.tensor.matmul(
                        ps,
                        lhsT=att_n[c][:, st * 128:(st + 1) * 128],
                        rhs=wo_all[:, c, eb * 512:(eb + 1) * 512],
                        start=(c == 0), stop=(c == 3),
                        skip_group_check=True)
                _mm(4 * 512)
                if eb == 0:
                    state["o"] = ostage.tile([128, 2, 512], BF16,
                                             name="o_sb", tag="o_sb")
                nc.vector.tensor_copy(state["o"][:, eb], ps)
                if eb == 1:
                    nc.sync.dma_start(
                        out=out[st],
                        in_=state["o"].rearrange("p a b -> p (a b)"))
            units.append((("o", qb, s4, eb), fn))
        return units

    for args in [("qk", 0, 0), ("v", 0), ("v", 1), ("v", 2), ("v", 3),
                 ("qk", 1, 0), ("qk", 0, 1),
                 ("v", 4), ("v", 5), ("v", 6), ("v", 7),
                 ("qk", 2, 0), ("qk", 1, 1), ("qk", 0, 2),
                 ("v", 8), ("v", 9), ("v", 10), ("v", 11),
                 ("qk", 3, 0), ("qk", 2, 1), ("qk", 1, 2), ("qk", 0, 3),
                 ("v", 12), ("v", 13), ("v", 14), ("v", 15),
                 ("qk", 3, 1), ("qk", 2, 2), ("qk", 1, 3),
                 ("qk", 3, 2), ("qk", 2, 3), ("qk", 3, 3)]:
        if args[0] == "qk":
            filler.extend(qk_units(args[1], args[2]))
        else:
            filler.extend(v_units(args[1]))

    def attention_block(c, qb, last=False):
        flush_for({("qk", c, s, w) for s in range(qb + 1)
                   for w in ("q", "k")})
        last_kt = 4 * qb + 3
        att_ps = [psatt.tile([HD + 1, 512], F32, name="att_ps",
                             tag="att_ps") for _ in range(2)]

        def av(kt, pt, cs, ready):
            flush_for({("v", kt)})
            # the AV matmuls can't start before their exp (+ causal mask)
            # lands; spend filler to keep the in-order PE stream busy
            fill_until(ready)
            for u in range(2):
                nc.tensor.matmul(
                    att_ps[u][:, cs:512],
                    lhsT=vP[kt][:, 2 * c + u, :],
                    rhs=pt[:, u, cs:512],
                    start=(kt == 0), stop=(kt == last_kt),
                    skip_group_check=True)
                _mm(512 - cs)
                T["pe"] += 30.0          # vP ldweights exposure

        pending = []
        act_done = {}
        for kt in range(last_kt + 1):
            diag = kt >= 4 * qb
            cs = 128 * kt - 512 * qb if diag else 0
            if kt >= 2:
                # s_ps ring (2 bufs = 2 kt-steps) reuse waits exp(kt-2)
                fill_until(act_done[kt - 2])
            s_ps = pst.tile([128, 2, 512], F32, name="s_ps", tag="s_ps")
            for u in range(2):
                po = u * HD
                nc.tensor.matmul(
                    s_ps[:, u, cs:512],
                    lhsT=kT_sb[c][po:po + HD, kt * 128:(kt + 1) * 128],
                    rhs=qT_sb[c][po:po + HD,
                                 qb * 512 + cs:(qb + 1) * 512],
                    start=True, stop=True, skip_group_check=True,
                    tile_position=(po, 0))
            _mm(512 - cs)          # concurrent row-tile pair
            T["pe"] += 100.0       # kT ldweights exposure
            # one 1024-wide exp covers both heads (stale cols in the
            # diag region are exp'd too but never read)
            pt = ptp.tile([128, 2, 512], BF16, name="pt", tag="pt")
            nc.scalar.activation(
                pt.rearrange("p a b -> p (a b)"),
                s_ps.rearrange("p a b -> p (a b)"), EXP, scale=0.125)
            _act(1024)
            if diag:
                for u in range(2):
                    # zero invalid (k > q) inside the 128-col triangle
                    # subtile; columns beyond it are fully valid
                    nc.gpsimd.affine_select(
                        out=pt[:, u, cs:cs + 128],
                        in_=pt[:, u, cs:cs + 128],
                        compare_op=GE, fill=0.0,
                        base=0, channel_multiplier=-1,
                        pattern=[[1, 128]])
            act_done[kt] = T["act"] + (500.0 if diag else 100.0)
            pending.append((kt, pt, cs, act_done[kt]))
            if len(pending) > 2:
                av(*pending.pop(0))
        while pending:
            av(*pending.pop(0))
        # normalize: reciprocal reads the denominator row straight from
        # PSUM; the two heads' chains are interleaved so DVE and gpsimd
        # pipeline instead of serializing
        r_sbs, rb_sbs = [], []
        for u in range(2):
            l_sb = smalls.tile([1, 512], F32, name="l_sb", tag=f"l_sb{u}")
            nc.vector.tensor_copy(l_sb, att_ps[u][HD:HD + 1, :])
            r_sb = smalls.tile([1, 512], F32, name="r_sb", tag=f"r_sb{u}")
            nc.vector.reciprocal_approx_fast(out=r_sb, in_=l_sb)
            r_sbs.append(r_sb)
        for u in range(2):
            rb_sb = smalls.tile([HD, 512], F32, name="rb_sb",
                                tag=f"rb_sb{u}")
            nc.gpsimd.partition_broadcast(rb_sb, r_sbs[u])
            rb_sbs.append(rb_sb)
        for u in range(2):
            po = u * HD
            nc.vector.tensor_mul(
                att_n[c][po:po + HD, qb * 512:(qb + 1) * 512],
                att_ps[u][0:HD, :], rb_sbs[u])
        if last:
            # final block: every remaining filler unit is independent of
            # this normalize -- burn it all now so the PE stays busy (and
            # the HAM clock stays warm) while the chain drains
            while filler:
                _emit_next_filler()
        else:
            fill_until(T["act"])

    # ---- wavefront over anti-diagonals with paced filler; hold back a
    # few o-units so the final (filler-starved) diagonal block can
    # still overlap its exp stream ----
    reserve = []
    done_qb = [0, 0, 0, 0]
    for d in range(7):
        for cc in range(3, -1, -1):
            qb = d - cc
            if not (0 <= qb <= 3):
                continue
            if d == 6:
                filler.extend(reserve)
                reserve = []
            attention_block(cc, qb, last=(d == 6))
            done_qb[qb] += 1
            if done_qb[qb] == 4:
                for s4 in range(4):
                    units = o_units(qb, s4)
                    if (qb, s4) in ((1, 3), (2, 2), (2, 3)):
                        reserve.extend(units)
                    else:
                        filler.extend(units)
    filler.extend(reserve)
    while filler:
        _emit_next_filler()

    ostage.release()
    smalls.release()
    ptp.release()
    pp.release()
    psatt.release()
    pst.release()
    persist1.release()


def _build():
    if "nc" in _CACHE:
        return _CACHE["nc"]
    nc = bacc.Bacc()
    with tile.TileContext(nc) as tc:
        _emit(nc, tc)
    nc.compile()
    _CACHE["nc"] = nc
    return nc


def _bf16(a):
    return np.ascontiguousarray(a.astype(ml_dtypes.bfloat16))


def _make_in_maps(inputs):
    x = np.asarray(inputs["x"], dtype=np.float32)
    Wq = np.asarray(inputs["Wq"], dtype=np.float32)
    Wk = np.asarray(inputs["Wk"], dtype=np.float32)
    Wv = np.asarray(inputs["Wv"], dtype=np.float32)
    Wo = np.asarray(inputs["Wo"], dtype=np.float32)
    in_maps = []
    for core in range(8):
        b, g = core // 2, core % 2
        cols = slice(g * G, (g + 1) * G)
        # xp[sb, p, ec, j] = x[b][sb*512 + j, ec*128 + p]
        xpk = x[b].T.reshape(EC, 128, SB, 512).transpose(2, 1, 0, 3)
        # wq[c, p, ec, m] = Wq[ec*128 + p, cols][c*128 + m]
        wqk = Wq[:, cols].reshape(EC, 128, 4, 128).transpose(2, 1, 0, 3)
        wkk = Wk[:, cols].reshape(EC, 128, 4, 128).transpose(2, 1, 0, 3)
        # wv[p, ec, j] = Wv[ec*128 + p, cols][j]
        wvk = Wv[:, cols].reshape(EC, 128, G).transpose(1, 0, 2)
        # wo[p, c, e] = Wo[cols][c*128 + p, e]
        wok = Wo[cols, :].reshape(4, 128, E).transpose(1, 0, 2)
        in_maps.append({
            "xp": _bf16(xpk),
            "wq": _bf16(wqk),
            "wk": _bf16(wkk),
            "wv": _bf16(wvk),
            "wo": _bf16(wok),
            "c_ones": np.ones((128, NH), dtype=ml_dtypes.bfloat16),
        })
    return in_maps


def kernel(x, Wq, Wk, Wv, Wo):
    nc = _build()
    in_maps = _make_in_maps(dict(x=x, Wq=Wq, Wk=Wk, Wv=Wv, Wo=Wo))
    res = run_bass_kernel_spmd(nc, in_maps, core_ids=list(range(8)))
    out = np.zeros((4, S, E), dtype=np.float32)
    for core in range(8):
        out[core // 2] += res.results[core]["out"].reshape(
            S, E).astype(np.float32)
    return out


if __name__ == "__main__":
    rng = np.random.default_rng(0)
    x = rng.standard_normal((4, S, E), dtype=np.float32)
    sc = 1.0 / np.sqrt(E)
    Wq = rng.standard_normal((E, E), dtype=np.float32) * sc
    Wk = rng.standard_normal((E, E), dtype=np.float32) * sc
    Wv = rng.standard_normal((E, E), dtype=np.float32) * sc
    Wo = rng.standard_normal((E, E), dtype=np.float32) * sc
    o = kernel(x, Wq, Wk, Wv, Wo)
    print("out", o.shape, o.dtype, np.abs(o).mean())


# revision 19
# speedup vs baseline: 1.0020x; 1.0020x over previous
"""Causal self-attention on 8 trn2 NeuronCores.

Sharding: core = (batch b, head-group g) with b in 0..3, g in 0..1.
Each core computes, for its batch and its 8 heads (512 of 1024 embed dims):
  QT/KT projections stored transposed [e', s] (e' on partitions)
  V stored [s, e'] with a ones-column appended per head
  S^T[k, q] = K_h Q_h^T      (scores transposed; k on partitions)
  P^T = exp(S^T / 8)         (no max-subtraction; scores are O(1))
  causal zeroing of P^T via gpsimd affine_select on the 128-col
  triangle subtile of diagonal tiles
  att'^T[d, q] = sum_k V'_h[k, d] P^T[k, q]   (row 64 = softmax denom l)
  att_n^T = att'^T[0:64] * (1/l)  (gpsimd partition_broadcast of 1/l)
  out_partial = att_n^T.T @ Wo[rows_g, :]
Host sums the two g-partials per batch (partials are bf16; sum in f32).

All matmuls run in bfloat16.  x is cast to bf16 on the host, packed so
every input DMA moves >=2KB per partition row (8KB for x/wv/wo), and
kept resident in SBUF.

Scheduling: per kt step the two heads' score matmuls share ONE
standalone full-row LDWEIGHTS (their stacked kT is contiguous in SBUF;
the matmuls are emitted non-self-loading) and write the 2 banks of one
[128, 2, 512] PSUM tile; a SINGLE 1024-wide ACTIVATE exps both heads
(amortizing the ~300ns ACT fixed cost 2x, full width -- stale
diag-triangle PSUM columns are exp'd but never read).  The s_ps ring
stays 2-deep (4 banks) so the Scalar engine never idles waiting for
score matmuls.  Projection / output-projection matmul groups are
interleaved INTO the attention stream by a deficit-paced scheduler
that models PE and ACT clocks; a few output-projection groups are held
back as filler for the final (filler-starved) diagonal block.  The AV
matmuls run two kt behind their score matmuls (software pipelining) so
the in-order PE stream never blocks on a just-issued exp.
"""
import sys

if "/opt/trn_rl_repo" not in sys.path:
    sys.path.insert(0, "/opt/trn_rl_repo")

import numpy as np
import ml_dtypes

import concourse.bacc as bacc
import concourse.mybir as mybir
import concourse.tile as tile
from concourse.bass_utils import run_bass_kernel_spmd

S = 2048          # sequence length
E = 1024          # embed dim
G = 512           # per-core head-group width (8 heads x 64)
HD = 64           # head dim
NH = 8            # heads per core
EC = E // 128     # 8 E-chunks
ST = S // 128     # 16 s-tiles
SB = S // 512     # 4 s-blocks
F32 = mybir.dt.float32
BF16 = mybir.dt.bfloat16
EXP = mybir.ActivationFunctionType.Exp
GE = mybir.AluOpType.is_ge

_CACHE = {}


def _emit(nc, tc):
    xp = nc.declare_dram_parameter("xp", [SB, 128, EC, 512], BF16,
                                   isOutput=False)
    wq = nc.declare_dram_parameter("wq", [4, 128, EC, 128], BF16,
                                   isOutput=False)
    wk = nc.declare_dram_parameter("wk", [4, 128, EC, 128], BF16,
                                   isOutput=False)
    wv = nc.declare_dram_parameter("wv", [128, EC, 512], BF16,
                                   isOutput=False)
    wo = nc.declare_dram_parameter("wo", [128, 4, E], BF16, isOutput=False)
    c_ones = nc.declare_dram_parameter("c_ones", [128, NH], BF16,
                                       isOutput=False)
    out = nc.declare_dram_parameter("out", [ST, 128, E], BF16,
                                    isOutput=True)

    # ---- long-lived SBUF state ----
    persist1 = tc.alloc_tile_pool(name="persist1", bufs=1, side="right")
    xall = persist1.tile([128, SB, EC, 512], BF16, name="xall", tag="xall")
    qT_sb, kT_sb = [], []
    for c in range(4):
        qT_sb.append(persist1.tile([128, S], BF16, name=f"qT{c}",
                                   tag=f"qT{c}"))
        kT_sb.append(persist1.tile([128, S], BF16, name=f"kT{c}",
                                   tag=f"kT{c}"))
    vP = [persist1.tile([128, NH, HD + 1], BF16, name=f"vP{st}",
                        tag=f"vP{st}") for st in range(ST)]
    att_n = [persist1.tile([128, S], BF16, name=f"attn{c}", tag=f"attn{c}")
             for c in range(4)]
    ones_sb = persist1.tile([128, NH], BF16, name="ones_sb", tag="ones_sb")
    wq_all = persist1.tile([128, 4, EC, 128], BF16, name="wq_all",
                           tag="wq_all")
    wk_all = persist1.tile([128, 4, EC, 128], BF16, name="wk_all",
                           tag="wk_all")
    wv_all = persist1.tile([128, EC, 512], BF16, name="wv_all",
                           tag="wv_all")
    wo_all = persist1.tile([128, 4, E], BF16, name="wo_all", tag="wo_all")
    wqk_sb = {("q", c): wq_all[:, c] for c in range(4)}
    wqk_sb.update({("k", c): wk_all[:, c] for c in range(4)})

    # ---- input DMAs, ordered so early compute unblocks first; every
    # transfer moves >=2KB per partition row ----
    nc.sync.dma_start(out=wq_all[:, 0], in_=wq[0])
    nc.sync.dma_start(out=wk_all[:, 0], in_=wk[0])
    nc.sync.dma_start(out=xall[:, 0], in_=xp[0])
    nc.sync.dma_start(out=wv_all, in_=wv[:, :, :])
    nc.sync.dma_start(out=ones_sb, in_=c_ones[:, :])
    nc.sync.dma_start(out=xall[:, 1], in_=xp[1])
    for c in range(1, 4):
        nc.sync.dma_start(out=wq_all[:, c], in_=wq[c])
        nc.sync.dma_start(out=wk_all[:, c], in_=wk[c])
    nc.sync.dma_start(out=xall[:, 2], in_=xp[2])
    nc.sync.dma_start(out=xall[:, 3], in_=xp[3])
    nc.sync.dma_start(out=wo_all, in_=wo[:, :, :])

    # ---- PSUM pools: 2x2 (s_ps ring) + 2 (att) + 2 (proj) = 8 banks ----
    pst = tc.alloc_tile_pool(name="pst", bufs=2, space="PSUM")
    psatt = tc.alloc_tile_pool(name="psatt", bufs=2, space="PSUM")
    pp = tc.alloc_tile_pool(name="pp", bufs=2, space="PSUM")
    ptp = tc.alloc_tile_pool(name="ptp", bufs=4)
    smalls = tc.alloc_tile_pool(name="smalls", bufs=2)
    ostage = tc.alloc_tile_pool(name="ostage", bufs=2)

    # warm-up: junk matmuls on a zeroed scratch tile keep the PE busy
    # through the startup DMA window so the HAM clock gate reaches full
    # rate before the first real projection group (results unread;
    # the first real start=True group overwrites the psum banks)
    warm = persist1.tile([128, 512], BF16, name="warm", tag="warm")
    nc.vector.memset(warm, 0.0)
    for _ in range(14):
        wps = pp.tile([128, 512], F32, name="ps_proj", tag="ps_proj")
        nc.tensor.matmul(wps, lhsT=warm[:, 0:128], rhs=warm,
                         start=True, stop=True, skip_group_check=True)

    # ---- stall-driven scheduler ----
    # T["pe"] / T["act"] model the two engines' busy-until times over the
    # emitted in-order streams.  Filler (projection / output-projection
    # half-groups, ~870ns of PE work each) is emitted exactly where the
    # in-order PE stream would otherwise stall waiting for an exp.
    T = {"pe": 0.0, "act": 0.0}
    emitted = set()
    filler = []   # list of (uid, closure)

    def _mm(rows):
        T["pe"] += rows * 0.425

    def _act(width):
        T["act"] = max(T["act"], T["pe"]) + width * 0.833 + 295.0

    def _emit_next_filler():
        uid, fn = filler.pop(0)
        fn()
        emitted.add(uid)

    def flush_for(need):
        while need - emitted:
            _emit_next_filler()

    def fill_until(t):
        while T["pe"] < t and filler:
            _emit_next_filler()

    def _proj_halves(uid, lhs_of_ec, rhs_of_ec, finish):
        # one [128,512] psum accumulation group as two filler halves
        state = {}

        def fn_a():
            state["ps"] = pp.tile([128, 512], F32, name="ps_proj",
                                  tag="ps_proj")
            for ec in range(4):
                nc.tensor.matmul(
                    state["ps"], lhsT=lhs_of_ec(ec), rhs=rhs_of_ec(ec),
                    start=(ec == 0), stop=False, skip_group_check=True)
            _mm(4 * 512)

        def fn_b():
            for ec in range(4, EC):
                nc.tensor.matmul(
                    state["ps"], lhsT=lhs_of_ec(ec), rhs=rhs_of_ec(ec),
                    start=False, stop=(ec == EC - 1), skip_group_check=True)
            _mm(4 * 512)
            finish(state["ps"])
        return [(uid + ("a",), fn_a), (uid, fn_b)]

    def qk_units(c, sb_i):
        units = []
        for wname, dest in (("q", qT_sb), ("k", kT_sb)):
            def finish(ps, dest=dest):
                nc.vector.tensor_copy(
                    dest[c][:, sb_i * 512:(sb_i + 1) * 512], ps)
            units.extend(_proj_halves(
                ("qk", c, sb_i, wname),
                lambda ec, wname=wname: wqk_sb[(wname, c)][:, ec, :],
                lambda ec: xall[:, sb_i, ec, :],
                finish))
        return units

    def v_units(st):
        def finish(ps):
            nc.vector.tensor_copy(vP[st][:, :, 0:HD],
                                  ps.rearrange("p (h d) -> p h d", h=NH))
            nc.vector.tensor_copy(vP[st][:, :, HD], ones_sb)
        c0 = (st % 4) * 128
        return _proj_halves(
            ("v", st),
            lambda ec: xall[:, st // 4, ec, c0:c0 + 128],
            lambda ec: wv_all[:, ec],
            finish)

    def o_units(qb, s4):
        st = qb * 4 + s4
        state = {}
        units = []
        for eb in range(2):
            def fn(eb=eb):
                ps = pp.tile([128, 512], F32, name="ps_o", tag="ps_proj")
                for c in range(4):
                    nc.tensor.matmul(
                        ps,
                        lhsT=att_n[c][:, st * 128:(st + 1) * 128],
                        rhs=wo_all[:, c, eb * 512:(eb + 1) * 512],
                        start=(c == 0), stop=(c == 3),
                        skip_group_check=True)
                _mm(4 * 512)
                if eb == 0:
                    state["o"] = ostage.tile([128, 2, 512], BF16,
                                             name="o_sb", tag="o_sb")
                nc.vector.tensor_copy(state["o"][:, eb], ps)
                if eb == 1:
                    nc.sync.dma_start(
                        out=out[st],
                        in_=state["o"].rearrange("p a b -> p (a b)"))
            units.append((("o", qb, s4, eb), fn))
        return units

    for args in [("qk", 0, 0), ("v", 0), ("v", 1), ("v", 2), ("v", 3),
                 ("qk", 1, 0), ("qk", 0, 1),
                 ("v", 4), ("v", 5), ("v", 6), ("v", 7),
                 ("qk", 2, 0), ("qk", 1, 1), ("qk", 0, 2),
                 ("v", 8), ("v", 9), ("v", 10), ("v", 11),
                 ("qk", 3, 0), ("qk", 2, 1), ("qk", 1, 2), ("qk", 0, 3),
                 ("v", 12), ("v", 13), ("v", 14), ("v", 15),
                 ("qk", 3, 1), ("qk", 2, 2), ("qk", 1, 3),
                 ("qk", 3, 2), ("qk", 2, 3), ("qk", 3, 3)]:
        if args[0] == "qk":
            filler.extend(qk_units(args[1], args[2]))
        else:
            filler.extend(v_units(args[1]))

    def attention_block(c, qb, last=False):
        flush_for({("qk", c, s, w) for s in range(qb + 1)
                   for w in ("q", "k")})
        last_kt = 4 * qb + 3
        att_ps = [psatt.tile([HD + 1, 512], F32, name="att_ps",
                             tag="att_ps") for _ in range(2)]

        def av(kt, pt, cs, ready):
            flush_for({("v", kt)})
            # the AV matmuls can't start before their exp (+ causal mask)
            # lands; spend filler to keep the in-order PE stream busy
            fill_until(ready)
            for u in range(2):
                nc.tensor.matmul(
                    att_ps[u][:, cs:512],
                    lhsT=vP[kt][:, 2 * c + u, :],
                    rhs=pt[:, u, cs:512],
                    start=(kt == 0), stop=(kt == last_kt),
                    skip_group_check=True)
                _mm(512 - cs)
                T["pe"] += 30.0          # vP ldweights exposure

        pending = []
        act_done = {}
        for kt in range(last_kt + 1):
            diag = kt >= 4 * qb
            cs = 128 * kt - 512 * qb if diag else 0
            if kt >= 2:
                # s_ps ring (2 bufs = 2 kt-steps) reuse waits exp(kt-2)
                fill_until(act_done[kt - 2])
            s_ps = pst.tile([128, 2, 512], F32, name="s_ps", tag="s_ps")
            for u in range(2):
                po = u * HD
                nc.tensor.matmul(
                    s_ps[:, u, cs:512],
                    lhsT=kT_sb[c][po:po + HD, kt * 128:(kt + 1) * 128],
                    rhs=qT_sb[c][po:po + HD,
                                 qb * 512 + cs:(qb + 1) * 512],
                    start=True, stop=True, skip_group_check=True,
                    tile_position=(po, 0))
            _mm(512 - cs)          # concurrent row-tile pair
            T["pe"] += 100.0       # kT ldweights exposure
            # one 1024-wide exp covers both heads (stale cols in the
            # diag region are exp'd too but never read)
            pt = ptp.tile([128, 2, 512], BF16, name="pt", tag="pt")
            nc.scalar.activation(
                pt.rearrange("p a b -> p (a b)"),
                s_ps.rearrange("p a b -> p (a b)"), EXP, scale=0.125)
            _act(1024)
            if diag:
                for u in range(2):
                    # zero invalid (k > q) inside the 128-col triangle
                    # subtile; columns beyond it are fully valid
                    nc.gpsimd.affine_select(
                        out=pt[:, u, cs:cs + 128],
                        in_=pt[:, u, cs:cs + 128],
                        compare_op=GE, fill=0.0,
                        base=0, channel_multiplier=-1,
                        pattern=[[1, 128]])
            act_done[kt] = T["act"] + (500.0 if diag else 100.0)
            pending.append((kt, pt, cs, act_done[kt]))
            if len(pending) > 2:
                av(*pending.pop(0))
        while pending:
            av(*pending.pop(0))
        if last:
            # emit a couple of reserve units BEFORE the normalize so
            # their att_n reads depend on older writes (not this
            # block's), keeping the PE busy through the chain below
            for _ in range(min(2, len(filler))):
                _emit_next_filler()
        # normalize: the two heads' chains are interleaved so DVE and
        # gpsimd pipeline instead of serializing
        r_sbs, rb_sbs = [], []
        for u in range(2):
            l_sb = smalls.tile([1, 512], F32, name="l_sb", tag=f"l_sb{u}")
            nc.vector.tensor_copy(l_sb, att_ps[u][HD:HD + 1, :])
            r_sb = smalls.tile([1, 512], F32, name="r_sb", tag=f"r_sb{u}")
            nc.vector.reciprocal_approx_fast(out=r_sb, in_=l_sb)
            r_sbs.append(r_sb)
        for u in range(2):
            rb_sb = smalls.tile([HD, 512], F32, name="rb_sb",
                                tag=f"rb_sb{u}")
            nc.gpsimd.partition_broadcast(rb_sb, r_sbs[u])
            rb_sbs.append(rb_sb)
        for u in range(2):
            po = u * HD
            nc.vector.tensor_mul(
                att_n[c][po:po + HD, qb * 512:(qb + 1) * 512],
                att_ps[u][0:HD, :], rb_sbs[u])
        if last:
            # final block: every remaining filler unit is independent of
            # this normalize -- burn it all now so the PE stays busy (and
            # the HAM clock stays warm) while the chain drains
            while filler:
                _emit_next_filler()
        else:
            fill_until(T["act"])

    # ---- wavefront over anti-diagonals with paced filler; hold back a
    # few o-units so the final (filler-starved) diagonal block can
    # still overlap its exp stream ----
    reserve = []
    done_qb = [0, 0, 0, 0]
    for d in range(7):
        for cc in range(3, -1, -1):
            qb = d - cc
            if not (0 <= qb <= 3):
                continue
            if d == 6:
                filler.extend(reserve)
                reserve = []
            attention_block(cc, qb, last=(d == 6))
            done_qb[qb] += 1
            if done_qb[qb] == 4:
                for s4 in range(4):
                    units = o_units(qb, s4)
                    if (qb, s4) in ((1, 3), (2, 2), (2, 3)):
                        reserve.extend(units)
                    else:
                        filler.extend(units)
    filler.extend(reserve)
    while filler:
        _emit_next_filler()

    ostage.release()
    smalls.release()
    ptp.release()
    pp.release()
    psatt.release()
    pst.release()
    persist1.release()


def _build():
    if "nc" in _CACHE:
        return _CACHE["nc"]
    nc = bacc.Bacc()
    with tile.TileContext(nc) as tc:
        _emit(nc, tc)
    nc.compile()
    _CACHE["nc"] = nc
    return nc


def _bf16(a):
    return np.ascontiguousarray(a.astype(ml_dtypes.bfloat16))


def _make_in_maps(inputs):
    x = np.asarray(inputs["x"], dtype=np.float32)
    Wq = np.asarray(inputs["Wq"], dtype=np.float32)
    Wk = np.asarray(inputs["Wk"], dtype=np.float32)
    Wv = np.asarray(inputs["Wv"], dtype=np.float32)
    Wo = np.asarray(inputs["Wo"], dtype=np.float32)
    in_maps = []
    for core in range(8):
        b, g = core // 2, core % 2
        cols = slice(g * G, (g + 1) * G)
        # xp[sb, p, ec, j] = x[b][sb*512 + j, ec*128 + p]
        xpk = x[b].T.reshape(EC, 128, SB, 512).transpose(2, 1, 0, 3)
        # wq[c, p, ec, m] = Wq[ec*128 + p, cols][c*128 + m]
        wqk = Wq[:, cols].reshape(EC, 128, 4, 128).transpose(2, 1, 0, 3)
        wkk = Wk[:, cols].reshape(EC, 128, 4, 128).transpose(2, 1, 0, 3)
        # wv[p, ec, j] = Wv[ec*128 + p, cols][j]
        wvk = Wv[:, cols].reshape(EC, 128, G).transpose(1, 0, 2)
        # wo[p, c, e] = Wo[cols][c*128 + p, e]
        wok = Wo[cols, :].reshape(4, 128, E).transpose(1, 0, 2)
        in_maps.append({
            "xp": _bf16(xpk),
            "wq": _bf16(wqk),
            "wk": _bf16(wkk),
            "wv": _bf16(wvk),
            "wo": _bf16(wok),
            "c_ones": np.ones((128, NH), dtype=ml_dtypes.bfloat16),
        })
    return in_maps


def kernel(x, Wq, Wk, Wv, Wo):
    nc = _build()
    in_maps = _make_in_maps(dict(x=x, Wq=Wq, Wk=Wk, Wv=Wv, Wo=Wo))
    res = run_bass_kernel_spmd(nc, in_maps, core_ids=list(range(8)))
    out = np.zeros((4, S, E), dtype=np.float32)
    for core in range(8):
        out[core // 2] += res.results[core]["out"].reshape(
            S, E).astype(np.float32)
    return out


if __name__ == "__main__":
    rng = np.random.default_rng(0)
    x = rng.standard_normal((4, S, E), dtype=np.float32)
    sc = 1.0 / np.sqrt(E)
    Wq = rng.standard_normal((E, E), dtype=np.float32) * sc
    Wk = rng.standard_normal((E, E), dtype=np.float32) * sc
    Wv = rng.standard_normal((E, E), dtype=np.float32) * sc
    Wo = rng.standard_normal((E, E), dtype=np.float32) * sc
    o = kernel(x, Wq, Wk, Wv, Wo)
    print("out", o.shape, o.dtype, np.abs(o).mean())
